# revision 74
# baseline (speedup 1.0000x reference)
"""Trainium2 Bass kernel for nn_BasicGroupCNN12 (SE(2) group CNN).

Strategy (8 NeuronCores):
  - Shard: 2 batch images x 4 spatial quadrants (64x64) = 8 cores, each core
    computing its quadrant with a shrinking redundant margin (no halo
    exchange); quadrants + kernels are host-flipped so all cores run the
    identical program with the true image corner at local (0,0).
  - Conv layers use a 16-matmul EXACT-COVER packing of the 25x80 = 2000
    (offset, channel) contraction slots (15x K=128 + 1x K=80, zero waste):
    four SBUF tiles hold shifted activation copies
      A[128] = 80ch@+0 | 48ch@+1      (streamed at dy=0..4)
      B[128] = 32ch@+1 | 80ch@+2 | 16ch@+3
      C[128] = 64ch@+3 | 64ch@+4
      E[80]  = 16ch (64-79) at row shifts dy=0..4 (streamed at dx=4)
    -> 0.8x the PE column-cycles of the previous 20-matmul scheme.
  - Shifted copies are FLAT contiguous SBUF->SBUF DMAs (a column shift is a
    flat offset; wrapped elements land in never-streamed columns), clamped
    at band ends so no copy depends on the next relu band.
  - Training-mode BatchNorm: per-chunk partial sums (DVE) -> [80,2] fp32
    AllGather + local sum (lower latency than AllReduce's mesh) -> rsqrt
    chain -> banded relu apply (one chunk lookahead) interleaved with the
    next layer's matmuls; a tiny 2-row first chunk restarts the PE ~4us
    earlier after each collective.  Collective in/out DMAs ride the gpsimd
    queue to stay clear of bulk-DMA semaphore traffic.
  - Tail: conv10 max-tree level 1 interleaved with conv10 chunks, rest under
    the stats collective; final 1x1 conv via a [128,8] block-selector so
    relu/sigmoid run as single wide ops.
  - Matmul operands fp16 (PSUM fp32): ~6e-4 final rel err.
"""

import os
import numpy as np

import concourse.bass as bass
import concourse.mybir as mybir
import concourse.tile as tile
from concourse import bacc
from concourse import bass_utils

F16 = mybir.dt.float16
F32 = mybir.dt.float32
NPDT = np.float16

O = 8        # orientations
C = 10       # channels per orientation (conv layers)
CF = 16      # channels in layer 10
E0 = 82      # lift output extent per core
EPS = 1e-5
NTOT = 2 * O * 128 * 128     # BN reduction element count per channel
AF = mybir.ActivationFunctionType
ALU = mybir.AluOpType


# ---------------------------------------------------------------------------
# Host-side weight/input preparation (pure numpy)
# ---------------------------------------------------------------------------

def _rot_matrices(k, n):
    c = (k - 1) / 2.0
    mats = np.zeros((n, k * k, k * k), np.float32)
    for m in range(n):
        th = 2.0 * np.pi * m / n
        co, si = np.cos(th), np.sin(th)
        for i in range(k):
            for j in range(k):
                di, dj = i - c, j - c
                sy = co * di + si * dj + c
                sx = -si * di + co * dj + c
                i0, j0 = int(np.floor(sy)), int(np.floor(sx))
                fy, fx = sy - i0, sx - j0
                for ii, jj, w in ((i0, j0, (1 - fy) * (1 - fx)), (i0, j0 + 1, (1 - fy) * fx),
                                  (i0 + 1, j0, fy * (1 - fx)), (i0 + 1, j0 + 1, fy * fx)):
                    if 0 <= ii < k and 0 <= jj < k and w > 1e-8:
                        mats[m, i * k + j, ii * k + jj] = w
    return mats


def _prep_rotated_weights(lift_w, conv_w, w10):
    M7 = _rot_matrices(7, O)
    M5 = _rot_matrices(5, O)
    Wlift = np.zeros((O, C, 3, 7, 7), np.float32)          # [m,co,ci,dy,dx]
    for m in range(O):
        Wlift[m] = (lift_w.reshape(C, 3, 49) @ M7[m].T).reshape(C, 3, 7, 7)
    Wconv = []
    for L in range(9):
        W = conv_w[L]                                      # [co,ci,n,5,5]
        Wf = np.zeros((O, C, O, C, 5, 5), np.float32)      # [m,co,n,ci,dy,dx]
        for m in range(O):
            Wm = np.roll(W, m, axis=2)
            Wm = (Wm.reshape(C, C, O, 25) @ M5[m].T).reshape(C, C, O, 5, 5)
            Wf[m] = Wm.transpose(0, 2, 1, 3, 4)
        Wconv.append(Wf)
    W10mat = np.zeros((96, 128), np.float32)
    for m in range(O):
        Wm = np.roll(w10[:, :, :, 0, 0], m, axis=2)        # [co,ci,n]
        W10mat[:80, m * CF:(m + 1) * CF] = Wm.transpose(2, 1, 0).reshape(80, CF)
    return Wlift, Wconv, W10mat


def _pack_m16(Wd5):
    """Pack flipped rotated weights [dy][dx][K=80][M=80] into the 16-matmul
    exact-cover stationary sets (zero slot waste: 2000 = 15*128 + 80).

    Tile A slots: p<80: ch p @ dx+0 ; p>=80: ch p-80 (0-47) @ dx+1
    Tile B slots: p<32: ch 48+p @ dx+1 ; 32<=p<112: ch p-32 @ dx+2 ;
                  p>=112: ch p-112 (0-15) @ dx+3
    Tile C slots: p<64: ch 16+p @ dx+3 ; p>=64: ch p-64 (0-63) @ dx+4
    Tile E slots: p -> ch 64+p%16 @ (dy=p//16, dx=4)
    Per chunk: 5 dy x (A,B,C) at stream offset (dy, 0) + one E at (0, 4).
    """
    SABC = np.zeros((15, 128, 80), np.float32)
    SE = np.zeros((80, 80), np.float32)
    for dy in range(5):
        S = SABC[dy * 3 + 0]
        S[0:80] = Wd5[dy][0]
        S[80:128] = Wd5[dy][1][0:48]
        S = SABC[dy * 3 + 1]
        S[0:32] = Wd5[dy][1][48:80]
        S[32:112] = Wd5[dy][2]
        S[112:128] = Wd5[dy][3][0:16]
        S = SABC[dy * 3 + 2]
        S[0:64] = Wd5[dy][3][16:80]
        S[64:128] = Wd5[dy][4][0:64]
        SE[16 * dy:16 * dy + 16] = Wd5[dy][4][64:80]
    return SABC, SE


def _check_pack16(Wd5, SABC, SE):
    # reconstruct effective per-(dy, dx, K, M) weight usage and compare
    acc = np.zeros((5, 5, 80, 80), np.float32)
    for dy in range(5):
        S = SABC[dy * 3 + 0]
        acc[dy, 0] += S[0:80]
        acc[dy, 1][0:48] += S[80:128]
        S = SABC[dy * 3 + 1]
        acc[dy, 1][48:80] += S[0:32]
        acc[dy, 2] += S[32:112]
        acc[dy, 3][0:16] += S[112:128]
        S = SABC[dy * 3 + 2]
        acc[dy, 3][16:80] += S[0:64]
        acc[dy, 4][0:64] += S[64:128]
        acc[dy, 4][64:80] += SE[16 * dy:16 * dy + 16]
    ref = np.stack([np.stack([Wd5[dy][dx] for dx in range(5)]) for dy in range(5)])
    assert np.abs(acc - ref).max() < 1e-6, "M=16 packing mismatch"


def _group_mat(P, nch):
    idx = np.arange(P)
    Gm = (idx[:, None] % nch == idx[None, :] % nch).astype(np.float32)
    return Gm * (1.0 / NTOT)


def _slot_bcast():
    # B1 [80,128], B2 [80,112]: B[k, p] = 1 if slot p holds channel k
    B1 = np.zeros((80, 128), np.float32)
    B2 = np.zeros((80, 112), np.float32)
    for p in range(128):
        B1[p if p < 80 else p - 80, p] = 1.0
    for p in range(112):
        B2[48 + p if p < 32 else p - 32, p] = 1.0
    return B1, B2


def prep_in_maps(inputs):
    x = np.asarray(inputs['x'], np.float32)
    lift_w = np.asarray(inputs['lift_w'], np.float32)
    conv_w = np.asarray(inputs['conv_w'], np.float32)
    w10 = np.asarray(inputs['w10'], np.float32)
    wfinal = np.asarray(inputs['wfinal'], np.float32)

    Wlift, Wconv, W10mat = _prep_rotated_weights(lift_w, conv_w, w10)

    gb = np.zeros((80, 10, 2), np.float32)
    gb[:, 0, 0] = np.asarray(inputs['lift_g'], np.float32)[np.arange(80) % C]
    gb[:, 0, 1] = np.asarray(inputs['lift_b'], np.float32)[np.arange(80) % C]
    cg = np.asarray(inputs['conv_g'], np.float32)
    cb = np.asarray(inputs['conv_b'], np.float32)
    for L in range(9):
        gb[:, L + 1, 0] = cg[L][np.arange(80) % C]
        gb[:, L + 1, 1] = cb[L][np.arange(80) % C]
    gb10 = np.zeros((128, 2), np.float32)
    gb10[:, 0] = np.asarray(inputs['g10'], np.float32)[np.arange(128) % CF]
    gb10[:, 1] = np.asarray(inputs['b10'], np.float32)[np.arange(128) % CF]

    G = _group_mat(80, C)
    G10 = _group_mat(128, CF)
    # final 1x1 conv as a block-selector: out[b, n] = sum_c w[c]*m128[16b+c, n]
    wf = wfinal[0, :, 0, 0]
    wfin = np.zeros((128, 8), np.float32)
    wfin[np.arange(128), np.arange(128) // 16] = wf[np.arange(128) % 16]
    wfin = wfin.astype(NPDT)

    # per-(qy,qx) flipped weight variants
    wvar = {}
    for qy in range(2):
        for qx in range(2):
            fy = slice(None, None, -1) if qy else slice(None)
            fx = slice(None, None, -1) if qx else slice(None)
            Wl = Wlift[:, :, :, fy, fx]                    # [m,co,ci,dy,dx]
            liftWmat = np.zeros((192, 80), np.float32)     # 147 rows + K-pad
            liftWmat[:147] = (
                Wl.transpose(3, 4, 2, 0, 1).reshape(49, 3, 80).reshape(147, 80))
            W1 = np.zeros((9, 128, 15, 80), np.float32)   # [L, K, d, M]
            W2 = np.zeros((9, 80, 80), np.float32)        # E tile weights
            for L in range(9):
                Wf = Wconv[L][:, :, :, :, fy, fx]          # [m,co,n,ci,dy,dx]
                Wd5 = Wf.transpose(4, 5, 2, 3, 0, 1).reshape(5, 5, 80, 80)
                SABC, SE = _pack_m16(Wd5)
                if L == 0:
                    _check_pack16(Wd5, SABC, SE)
                W1[L] = SABC.transpose(1, 0, 2)
                W2[L] = SE
            wvar[(qy, qx)] = (liftWmat.astype(NPDT), W1.astype(NPDT),
                              W2.astype(NPDT))

    in_maps = []
    cores = []
    for b in range(2):
        for qy in range(2):
            for qx in range(2):
                cores.append((b, qy, qx))
                xf = x[b]
                if qy:
                    xf = xf[:, ::-1, :]
                if qx:
                    xf = xf[:, :, ::-1]
                win = xf[:, 0:85, 0:85]
                xw = np.pad(win, ((0, 0), (3, 0), (3, 0)), mode='reflect')
                col = np.zeros((192, E0 * E0), NPDT)       # 147 rows + K-pad
                for t in range(49):
                    dy, dx = t // 7, t % 7
                    col[t * 3:t * 3 + 3] = (
                        xw[:, dy:dy + E0, dx:dx + E0].reshape(3, -1))
                liftWmat, W1, W2 = wvar[(qy, qx)]
                in_maps.append({
                    'xcol1': np.ascontiguousarray(col[:96]),
                    'xcol2': np.ascontiguousarray(col[96:]),
                    'liftW1': np.ascontiguousarray(liftWmat[:96]),
                    'liftW2': np.ascontiguousarray(liftWmat[96:]),
                    'convW1': W1,
                    'convW2': W2,
                    'w10': W10mat.astype(NPDT),
                    'wfin': wfin,
                    'G': G,
                    'G10': G10,
                    'gb': gb,
                    'gb10': gb10,
                })
    return in_maps, cores


# ---------------------------------------------------------------------------
# Bass program
# ---------------------------------------------------------------------------

def build_program(no_cc=None, nconv=None, fold_max=True):
    if no_cc is None:
        no_cc = bool(os.environ.get("GK_NO_CC"))
    if nconv is None:
        nconv = int(os.environ.get("GK_NCONV", "9"))
    use_ag = os.environ.get("GK_CC", "ag") == "ag"
    nc = bacc.Bacc("TRN2", target_bir_lowering=False, debug=False,
                   enable_asserts=False, num_devices=8)

    d_xcol1 = nc.dram_tensor("xcol1", [96, E0 * E0], F16, kind="ExternalInput")
    d_xcol2 = nc.dram_tensor("xcol2", [96, E0 * E0], F16, kind="ExternalInput")
    d_liftW1 = nc.dram_tensor("liftW1", [96, 80], F16, kind="ExternalInput")
    d_liftW2 = nc.dram_tensor("liftW2", [96, 80], F16, kind="ExternalInput")
    d_convW1 = nc.dram_tensor("convW1", [9, 128, 15, 80], F16, kind="ExternalInput")
    d_convW2 = nc.dram_tensor("convW2", [9, 80, 80], F16, kind="ExternalInput")
    d_w10 = nc.dram_tensor("w10", [96, 128], F16, kind="ExternalInput")
    d_wfin = nc.dram_tensor("wfin", [128, 8], F16, kind="ExternalInput")
    d_G = nc.dram_tensor("G", [80, 80], F32, kind="ExternalInput")
    d_G10 = nc.dram_tensor("G10", [128, 128], F32, kind="ExternalInput")
    d_gb = nc.dram_tensor("gb", [80, 10, 2], F32, kind="ExternalInput")
    d_gb10 = nc.dram_tensor("gb10", [128, 2], F32, kind="ExternalInput")
    d_out = nc.dram_tensor("out", [1, 64 * 64], F32, kind="ExternalOutput")

    rg = [list(range(8))]

    with tile.TileContext(nc) as tc:
        with (
            tc.tile_pool(name="const", bufs=1) as constp,
            tc.tile_pool(name="acts", bufs=2) as actsp,
            tc.tile_pool(name="tails", bufs=1) as tailsp,
            tc.tile_pool(name="smallp", bufs=8) as smallp,
            tc.tile_pool(name="sbp", bufs=6) as sbp,
            tc.tile_pool(name="sqp", bufs=1) as sqp,
            tc.tile_pool(name="pscv", bufs=6, space="PSUM") as pscv,
            tc.tile_pool(name="psst", bufs=2, space="PSUM") as psst,
            tc.tile_pool(name="dram", bufs=2, space="DRAM") as dramp,
        ):
            # ---- sigmoid table preload (pays ACT_TABLE_LOAD during startup) ----
            epst = constp.tile([128, 1], F32, tag="eps")
            nc.vector.memset(epst[:], EPS)
            sigwarm = smallp.tile([1, 1], F32, tag="sigwarm")
            nc.scalar.activation(sigwarm[:], epst[0:1, 0:1], AF.Sigmoid)
            if os.environ.get("GK_WARMUP") and not no_cc:
                wu_in = dramp.tile([8, 1], F32, tag="cc_in")
                wu_out = dramp.tile([8, 1], F32, tag="cc_out")
                nc.gpsimd.dma_start(out=wu_in[:], in_=epst[0:8, 0:1])
                nc.gpsimd.collective_compute(
                    "AllReduce", ALU.add, replica_groups=rg,
                    ins=[wu_in[:].opt()], outs=[wu_out[:].opt()])

            # ---- load constants; xcol chunked so lift starts early ----
            r0 = 512 // E0                       # lift chunk rows (6)
            lift_chunks = [(y0, min(r0, E0 - y0)) for y0 in range(0, E0, r0)]
            # xcol is dead after the lift; borrow the tail-phase buffers
            # (tile framework inserts WAR sync on tag reuse)
            xc1 = tailsp.tile([96, E0 * E0], F16, tag="b0")
            xc2 = tailsp.tile([96, E0 * E0], F16, tag="m128")
            # first 5 lift-chunks worth of xcol, then the rest
            splits = [0, 5 * r0 * E0, E0 * E0]
            for a, b in zip(splits[:-1], splits[1:]):
                nc.sync.dma_start(out=xc1[:, a:b], in_=d_xcol1.ap()[:, a:b])
                nc.sync.dma_start(out=xc2[:, a:b], in_=d_xcol2.ap()[:, a:b])
            lw1 = constp.tile([96, 80], F16, tag="lw1")
            nc.sync.dma_start(out=lw1[:], in_=d_liftW1.ap())
            lw2 = constp.tile([96, 80], F16, tag="lw2")
            nc.sync.dma_start(out=lw2[:], in_=d_liftW2.ap())
            Gt = constp.tile([80, 80], F32, tag="G")
            nc.sync.dma_start(out=Gt[:], in_=d_G.ap())
            gbt = constp.tile([80, 10, 2], F32, tag="gb")
            nc.sync.dma_start(out=gbt[:], in_=d_gb.ap())
            cw1 = []
            cw2 = []
            for L in range(9):
                t1w = constp.tile([128, 15, 80], F16, tag=f"cw1_{L}")
                nc.sync.dma_start(out=t1w[:], in_=d_convW1.ap()[L])
                t2w = constp.tile([80, 80], F16, tag=f"cw2_{L}")
                nc.sync.dma_start(out=t2w[:], in_=d_convW2.ap()[L])
                cw1.append(t1w)
                cw2.append(t2w)
            G10t = constp.tile([128, 128], F32, tag="G10")
            nc.sync.dma_start(out=G10t[:], in_=d_G10.ap())
            gb10t = constp.tile([128, 2], F32, tag="gb10")
            nc.sync.dma_start(out=gb10t[:], in_=d_gb10.ap())
            w10t = constp.tile([96, 128], F16, tag="w10")
            nc.sync.dma_start(out=w10t[:], in_=d_w10.ap())
            wfint = constp.tile([128, 8], F16, tag="wfin")
            nc.sync.dma_start(out=wfint[:], in_=d_wfin.ap())

            def collective(v_ap, P):
                """Launch stats collective on [P,2]; returns fn() -> sbuf [P,2].

                cin/fetch DMAs ride the gpsimd queue so the CC doorbell's
                semaphore wait is not entangled with bulk DMA traffic."""
                cin = dramp.tile([P, 2], F32, tag="cc_in")
                nc.gpsimd.dma_start(out=cin[:], in_=v_ap)
                if no_cc:
                    cout = dramp.tile([P, 2], F32, tag="cc_out")
                    nc.gpsimd.dma_start(out=cout[:], in_=cin[:])

                    def fetch():
                        post = smallp.tile([P, 2], F32, tag="post")
                        nc.gpsimd.dma_start(out=post[:], in_=cout[:])
                        return post
                elif use_ag:
                    cout = dramp.tile([8, P, 2], F32, tag="cc_out")
                    nc.gpsimd.collective_compute(
                        "AllGather", ALU.bypass, replica_groups=rg,
                        ins=[cin[:].opt()], outs=[cout[:].opt()])

                    def fetch():
                        ga = smallp.tile([P, 2, 8], F32, tag="ga")
                        nc.gpsimd.dma_start(
                            out=ga[:], in_=cout[:].rearrange("g c s -> c s g"))
                        post = smallp.tile([P, 2], F32, tag="post")
                        nc.vector.tensor_reduce(
                            out=post[:], in_=ga[:],
                            axis=mybir.AxisListType.X, op=ALU.add)
                        return post
                else:
                    cout = dramp.tile([P, 2], F32, tag="cc_out")
                    nc.gpsimd.collective_compute(
                        "AllReduce", ALU.add, replica_groups=rg,
                        ins=[cin[:].opt()], outs=[cout[:].opt()])

                    def fetch():
                        post = smallp.tile([P, 2], F32, tag="post")
                        nc.gpsimd.dma_start(out=post[:], in_=cout[:])
                        return post
                return fetch

            def stats_chain(post, P, Gtile, gamma_ap, beta_ap):
                """post [P,2] raw sums -> (scl, bia) [P,1] each (SBUF)."""
                gps = psst.tile([P, 2], F32, tag="st")
                nc.tensor.matmul(gps[:], Gtile[:], post[:],
                                 start=True, stop=True)
                msq = smallp.tile([P, 1], F32, tag="msq")
                nc.scalar.square(msq[:], gps[:, 0:1])
                var = smallp.tile([P, 1], F32, tag="var")
                nc.vector.tensor_sub(var[:], gps[:, 1:2], msq[:])
                std = smallp.tile([P, 1], F32, tag="std")
                nc.scalar.activation(std[:], var[:], AF.Sqrt,
                                     bias=epst[0:P, 0:1])
                rstd = smallp.tile([P, 1], F32, tag="rstd")
                nc.vector.reciprocal(rstd[:], std[:])
                sclbia = sbp.tile([P, 2], F32, tag="sclbia")
                nc.vector.tensor_mul(sclbia[:, 0:1], rstd[:], gamma_ap)
                tb = smallp.tile([P, 1], F32, tag="tb")
                nc.vector.tensor_mul(tb[:], gps[:, 0:1], sclbia[:, 0:1])
                nc.vector.tensor_sub(sclbia[:, 1:2], beta_ap, tb[:])
                return sclbia

            # ================= layer emitters =================
            state = {}

            def emit_packed_layer(mms_fn, E_out, lidx, plain_out=False):
                """Emit one conv-like layer writing frame tiles (or plain buf).

                mms_fn(ps, y0, rr): accumulate matmuls for out rows [y0,y0+rr).
                Output written to tile-A primary frame at (2+y, 2+x) (or buf
                at (y, x) when plain_out).  Leaves state['A'..'E'] (or
                state['buf']) for the next layer.
                """
                r = 512 // E_out
                # tiny first chunk: needs only rows [0,6) applied, so the
                # PE restarts ~4us earlier after each stats collective
                chunks = [(0, 2)] + [(y0, min(r, E_out - y0))
                                     for y0 in range(2, E_out, r)]
                img_chunks = [c for c in chunks if c[0] < 64]
                mar_chunks = [c for c in chunks if c[0] >= 64]
                n_img = len(img_chunks)
                doff = 0 if plain_out else 2
                # next-layer input frame extent = E_out + 2 borders
                Fn = E_out + 2
                if plain_out:
                    dst = tailsp.tile([96, 64, 64], F16, tag="buf9")
                    nc.vector.memset(dst[64:96, :, :], 0.0)
                    state['buf'] = dst
                    tA = dst
                else:
                    tA = actsp.tile([128, Fn, Fn], F16, tag="tA")
                    tB = actsp.tile([128, Fn, Fn], F16, tag="tB")
                    tC = actsp.tile([128, Fn, Fn], F16, tag="tC")
                    tE = actsp.tile([80, Fn, Fn], F16, tag="tE")
                    state['A'], state['B'] = tA, tB
                    state['C'], state['E'] = tC, tE

                parts_s = smallp.tile([80, 16], F32, tag="pts")
                parts_q = smallp.tile([80, 16], F32, tag="ptq")

                def do_chunk(ci, y0, rr, stats):
                    ps = pscv.tile([80, rr, E_out], F32, tag="cv")
                    mms_fn(ps, y0, rr)
                    nc.scalar.copy(
                        tA[0:80, doff + y0:doff + y0 + rr,
                           doff:doff + E_out], ps[:])
                    if not plain_out:
                        # col reflect (raw): cols 0,1 <- 4,3 for these rows
                        rows = slice(doff + y0, doff + y0 + rr)
                        nc.vector.tensor_copy(tA[0:80, rows, 0:1],
                                              tA[0:80, rows, 4:5])
                        nc.vector.tensor_copy(tA[0:80, rows, 1:2],
                                              tA[0:80, rows, 3:4])
                        if ci == 1:
                            # row reflect (raw): rows 0,1 <- 4,3 full width
                            # (frame row 4 is first written by chunk 1)
                            nc.vector.tensor_copy(tA[0:80, 0:1, :],
                                                  tA[0:80, 4:5, :])
                            nc.vector.tensor_copy(tA[0:80, 1:2, :],
                                                  tA[0:80, 3:4, :])
                    if stats:
                        h = min(y0 + rr, 64) - y0
                        reg = tA[0:80, doff + y0:doff + y0 + h,
                                 doff:doff + 64]
                        nc.vector.tensor_reduce(
                            out=parts_s[:, ci:ci + 1], in_=reg,
                            axis=mybir.AxisListType.XY, op=ALU.add)
                        sqc = sqp.tile([80, 8, 64], F16, tag="sqc")
                        nc.vector.tensor_mul(sqc[0:80, 0:h, :], reg, reg)
                        nc.vector.tensor_reduce(
                            out=parts_q[:, ci:ci + 1],
                            in_=sqc[0:80, 0:h, :],
                            axis=mybir.AxisListType.XY, op=ALU.add)

                # flat views: a column shift s becomes a flat offset of s
                # elements, so every dup copy is one contiguous segment per
                # partition (single DMA packet instead of per-row packets).
                # Wrapped elements land only in cols >= Fn-4, never streamed.
                fA = tA[:].rearrange("p r c -> p (r c)") if not plain_out \
                    else None
                if not plain_out:
                    fB = tB[:].rearrange("p r c -> p (r c)")
                    fC = tC[:].rearrange("p r c -> p (r c)")
                    fE = tE[:].rearrange("p r c -> p (r c)")

                def dup_band(ra, rb):
                    # duplicate APPLIED rows [ra, rb) of the new frame into
                    # the shifted slots (partition-moving SBUF->SBUF DMAs),
                    # spread across queues.  Copy length is clamped to the
                    # band end so no dup reads into the next relu band; the
                    # missing tail lands in cols >= Fn-4, never streamed.
                    a0 = ra * Fn
                    b0 = rb * Fn

                    def flat(q, dstf, p0, p1, sp0, sft):
                        q.dma_start(
                            out=dstf[p0:p1, a0:b0 - sft],
                            in_=fA[sp0:sp0 + (p1 - p0), a0 + sft:b0])

                    flat(nc.scalar, fA, 80, 128, 0, 1)    # A g2: ch0-47 @+1
                    flat(nc.sync, fB, 32, 112, 0, 2)      # B g2: ch0-79 @+2
                    flat(nc.sync, fC, 64, 128, 0, 4)      # C g2: ch0-63 @+4
                    flat(nc.sync, fC, 0, 64, 16, 3)       # C g1: ch16-79 @+3
                    flat(nc.gpsimd, fB, 0, 32, 48, 1)     # B g1: ch48-79 @+1
                    flat(nc.gpsimd, fB, 112, 128, 0, 3)   # B g3: ch0-15 @+3
                    # E: row-shifted copies of ch64-79 (lags 4 applied rows)
                    ea, eb = max(0, ra - 4), max(0, rb - 4)
                    if rb >= Fn:
                        eb = Fn - 4          # final band: finish E fully
                    if eb > ea:
                        for g in range(5):
                            q = (nc.gpsimd, nc.sync, nc.scalar,
                                 nc.gpsimd, nc.scalar)[g]
                            q.dma_start(
                                out=fE[16 * g:16 * g + 16,
                                       ea * Fn:eb * Fn],
                                in_=fA[64:80,
                                       (ea + g) * Fn:(eb + g) * Fn])

                for ci, (y0, rr) in enumerate(img_chunks):
                    do_chunk(ci, y0, rr, True)
                # finalize local stats and launch the collective
                v = smallp.tile([80, 2], F32, tag="v")
                nc.vector.tensor_reduce(
                    out=v[:, 0:1], in_=parts_s[:, 0:n_img],
                    axis=mybir.AxisListType.X, op=ALU.add)
                nc.vector.tensor_reduce(
                    out=v[:, 1:2], in_=parts_q[:, 0:n_img],
                    axis=mybir.AxisListType.X, op=ALU.add)
                fetch = collective(v[:], 80)
                # margin chunks cover the collective latency on the PE
                for ci, (y0, rr) in enumerate(mar_chunks):
                    do_chunk(n_img + ci, y0, rr, False)
                # keep-warm dummy matmuls: keep the PE HAM throttle released
                # while waiting for the collective (no reader, no drain)
                post = fetch()
                sclbia = stats_chain(post, 80, Gt,
                                     gbt[:, lidx, 0:1], gbt[:, lidx, 1:2])
                if plain_out:
                    state['sb9'] = sclbia
                    return
                state['apply'] = (tA, dup_band, sclbia, Fn)

            def apply_rows(ra, rb):
                tA, dup_band, sclbia, Fn = state['apply']
                rb = min(rb, Fn)
                if rb <= ra:
                    return
                nc.scalar.activation(tA[0:80, ra:rb, :],
                                     tA[0:80, ra:rb, :], AF.Relu,
                                     bias=sclbia[:, 1:2],
                                     scale=sclbia[:, 0:1])
                dup_band(ra, rb)

            def apply_rows9(ra, rb):
                buf, sclbia = state['buf'], state['sb9']
                rb = min(rb, 64)
                if rb <= ra:
                    return
                nc.scalar.activation(buf[0:80, ra:rb, :],
                                     buf[0:80, ra:rb, :], AF.Relu,
                                     bias=sclbia[:, 1:2],
                                     scale=sclbia[:, 0:1])

            # ---- lift layer ----
            def lift_mms(ps, y0, rr):
                nc.tensor.matmul(ps[:], lw1[:],
                                 xc1[:, y0 * E0:(y0 + rr) * E0],
                                 start=True, stop=False)
                nc.tensor.matmul(ps[:], lw2[:],
                                 xc2[:, y0 * E0:(y0 + rr) * E0],
                                 start=False, stop=True)

            emit_packed_layer(lift_mms, E0, 0)

            # ---- conv layers 1..9 ----
            prev_apply = apply_rows
            for L in range(9):
                E_in = E0 - 2 * L            # input extent (incl borders = E_in+2)
                E_out = E_in - 2
                plain = (L == 8)
                sA, sB = state['A'], state['B']
                sC, sE = state['C'], state['E']
                w1, w2 = cw1[L], cw2[L]

                def conv_mms(ps, y0, rr, w1=w1, w2=w2, sA=sA, sB=sB,
                             sC=sC, sE=sE, E_out=E_out):
                    for d in range(15):
                        dy, j = d // 3, d % 3
                        src = (sA, sB, sC)[j]
                        nc.tensor.matmul(
                            ps[:], w1[:, d, :],
                            src[0:128, y0 + dy:y0 + dy + rr, 0:E_out],
                            start=(d == 0), stop=False)
                    nc.tensor.matmul(
                        ps[:], w2[:],
                        sE[0:80, y0:y0 + rr, 4:4 + E_out],
                        start=False, stop=True)

                # interleave previous layer's apply with this layer's chunks:
                # emit_packed_layer internally iterates chunks; we need the
                # apply of the PREVIOUS layer to be emitted piecewise before
                # each chunk.  Wrap mms_fn to emit apply pieces lazily.
                r = 512 // E_out
                need = {'done': 0}

                def conv_mms_gated(ps, y0, rr, conv_mms=conv_mms, r=r,
                                   need=need, L=L):
                    # ensure prev apply covers rows [0, y0+rr+4); after the
                    # first (tiny) band, apply one chunk ahead so dup DMAs
                    # batch over ~2-chunk bands
                    if prev_apply is not None:
                        want = y0 + rr + 4
                        if want > need['done']:
                            if need['done'] > 0:
                                want += rr
                            prev_apply(need['done'], want)
                            need['done'] = want
                    conv_mms(ps, y0, rr)

                emit_packed_layer(conv_mms_gated, E_out, L + 1,
                                  plain_out=plain)
                # finish any remaining apply rows of the previous layer
                if prev_apply is not None:
                    prev_apply(need['done'], E_in + 2)
                if plain:
                    prev_apply = apply_rows9
                else:
                    prev_apply = apply_rows
                if L + 1 >= nconv:
                    pass

            # ---- conv10 (1x1, 80 -> 128 partitions) ----
            act10 = tailsp.tile([128, 64, 64], F16, tag="act10")
            buf9 = state['buf']

            need9 = {'done': 0}

            def c10_mms(ps, y0, rr, src=buf9):
                want = y0 + rr
                if want > need9['done']:
                    prev_apply(need9['done'], want)
                    need9['done'] = want
                nc.tensor.matmul(ps[:], w10t[:], src[:, y0:y0 + rr, 0:64],
                                 start=True, stop=True)

            # conv10 layer: special emit (128 partitions, no margins)
            r = 512 // 64
            chunks10 = [(y0, 8) for y0 in range(0, 64, 8)]
            parts_s = smallp.tile([128, 16], F32, tag="pts10")
            parts_q = smallp.tile([128, 16], F32, tag="ptq10")
            a10 = act10[:].rearrange("p a b -> p (a b)")
            b0 = tailsp.tile([64, 4096], F16, tag="b0")
            for ci, (y0, rr) in enumerate(chunks10):
                ps = pscv.tile([128, rr, 64], F32, tag="cv")
                c10_mms(ps, y0, rr)
                nc.scalar.copy(act10[:, y0:y0 + rr, :], ps[:])
                nc.vector.tensor_reduce(
                    out=parts_s[:, ci:ci + 1],
                    in_=act10[:, y0:y0 + rr, :],
                    axis=mybir.AxisListType.XY, op=ALU.add)
                sqc = sqp.tile([128, 8, 64], F16, tag="sqc10")
                nc.gpsimd.tensor_mul(sqc[:, 0:rr, :],
                                     act10[:, y0:y0 + rr, :],
                                     act10[:, y0:y0 + rr, :])
                nc.vector.tensor_reduce(
                    out=parts_q[:, ci:ci + 1], in_=sqc[:, 0:rr, :],
                    axis=mybir.AxisListType.XY, op=ALU.add)
                # max-tree level 1 for this chunk's pixel block (the
                # remaining levels run under the stats collective)
                csl = slice(64 * y0, 64 * (y0 + rr))
                nc.sync.dma_start(out=b0[0:64, csl], in_=a10[64:128, csl])
                nc.vector.tensor_max(a10[0:64, csl], a10[0:64, csl],
                                     b0[0:64, csl])
            v10 = smallp.tile([128, 2], F32, tag="v10")
            nc.vector.tensor_reduce(
                out=v10[:, 0:1], in_=parts_s[:, 0:len(chunks10)],
                axis=mybir.AxisListType.X, op=ALU.add)
            nc.vector.tensor_reduce(
                out=v10[:, 1:2], in_=parts_q[:, 0:len(chunks10)],
                axis=mybir.AxisListType.X, op=ALU.add)
            fetch10 = collective(v10[:], 128)

            # ---- max tree levels 2-3 (level 1 ran per conv10 chunk) ----
            # in-place in act10 with one scratch tile to save SBUF; levels
            # split into halves so the copy/max ping-pong pipelines
            for p in (32, 16):
                for h in range(2):
                    cs = slice(2048 * h, 2048 * (h + 1))
                    nc.sync.dma_start(out=b0[0:p, cs],
                                      in_=a10[p:2 * p, cs])
                    nc.vector.tensor_max(a10[0:p, cs], a10[0:p, cs],
                                         b0[0:p, cs])
            # spread [16, 8*512] -> [128, 512]: partition 16*b+c holds
            # channel c, column block b (still under the AR)
            m128 = tailsp.tile([128, 512], F16, tag="m128")
            for b in range(8):
                q = (nc.scalar, nc.sync)[b % 2]
                q.dma_start(out=m128[16 * b:16 * b + 16, :],
                            in_=a10[0:16, 512 * b:512 * (b + 1)])

            post10 = fetch10()
            sb10 = stats_chain(post10, 128, G10t,
                               gb10t[:, 0:1], gb10t[:, 1:2])
            # relu(s*h+b) wide (gamma>0 commutes w/ max), one final matmul
            # with the block-selector weights, one sigmoid, one output DMA
            nc.scalar.activation(m128[:], m128[:], AF.Relu,
                                 bias=sb10[:, 1:2], scale=sb10[:, 0:1])
            fp = psst.tile([8, 512], F32, tag="st")
            nc.tensor.matmul(fp[:], wfint[:], m128[:],
                             start=True, stop=True)
            ob = smallp.tile([8, 512], F32, tag="outsb")
            nc.scalar.activation(ob[:], fp[:], AF.Sigmoid)
            nc.sync.dma_start(
                out=d_out.ap().rearrange("o (b n) -> (o b) n", b=8),
                in_=ob[:])

    nc.compile()
    return nc


_CACHED = {}


def _get_program(fold_max):
    key = ('nc', fold_max)
    if key not in _CACHED:
        _CACHED[key] = build_program(fold_max=fold_max)
    return _CACHED[key]


LAST = None


def kernel(**inputs):
    global LAST
    in_maps, cores = prep_in_maps(inputs)
    fold_max = bool((np.asarray(inputs['g10'], np.float32) > 0).all())
    nc = _get_program(fold_max)
    res = bass_utils.run_bass_kernel_spmd(
        nc, in_maps, core_ids=list(range(8)),
        trace=bool(os.environ.get("GK_TRACE")))
    LAST = res
    out = np.zeros((2, 1, 128, 128), np.float32)
    for (core, omap) in zip(cores, res.results):
        b, qy, qx = core
        q = np.asarray(omap['out'], np.float32).reshape(64, 64)
        if qy:
            q = q[::-1, :]
        if qx:
            q = q[:, ::-1]
        out[b, 0, qy * 64:(qy + 1) * 64, qx * 64:(qx + 1) * 64] = q
    return out



# revision 79
# speedup vs baseline: 1.0319x; 1.0319x over previous
"""Trainium2 Bass kernel for nn_BasicGroupCNN12 (SE(2) group CNN).

Strategy (8 NeuronCores):
  - Shard: 2 batch images x 4 spatial quadrants (64x64) = 8 cores, each core
    computing its quadrant with a shrinking redundant margin (no halo
    exchange); quadrants + kernels are host-flipped so all cores run the
    identical program with the true image corner at local (0,0).
  - Conv layers use a 16-matmul EXACT-COVER packing of the 25x80 = 2000
    (offset, channel) contraction slots (15x K=128 + 1x K=80, zero waste):
    four SBUF tiles hold shifted activation copies
      A[128] = 80ch@+0 | 48ch@+1      (streamed at dy=0..4)
      B[128] = 32ch@+1 | 80ch@+2 | 16ch@+3
      C[128] = 64ch@+3 | 64ch@+4
      E[80]  = 16ch (64-79) at row shifts dy=0..4 (streamed at dx=4)
    -> 0.8x the PE column-cycles of the previous 20-matmul scheme.
  - Shifted copies are FLAT contiguous SBUF->SBUF DMAs (a column shift is a
    flat offset; wrapped elements land in never-streamed columns), clamped
    at band ends so no copy depends on the next relu band.
  - Training-mode BatchNorm: per-chunk partial sums (DVE) -> [80,2] fp32
    AllGather + local sum (lower latency than AllReduce's mesh) -> rsqrt
    chain -> banded relu apply (one chunk lookahead) interleaved with the
    next layer's matmuls; a tiny 2-row first chunk restarts the PE ~4us
    earlier after each collective.  Collective in/out DMAs ride the gpsimd
    queue to stay clear of bulk-DMA semaphore traffic.
  - Tail: conv10 max-tree level 1 interleaved with conv10 chunks, rest under
    the stats collective; final 1x1 conv via a [128,8] block-selector so
    relu/sigmoid run as single wide ops.
  - Matmul operands fp16 (PSUM fp32): ~6e-4 final rel err.
"""

import os
import numpy as np

import concourse.bass as bass
import concourse.mybir as mybir
import concourse.tile as tile
from concourse import bacc
from concourse import bass_utils

F16 = mybir.dt.float16
F32 = mybir.dt.float32
NPDT = np.float16

O = 8        # orientations
C = 10       # channels per orientation (conv layers)
CF = 16      # channels in layer 10
E0 = 82      # lift output extent per core
EPS = 1e-5
NTOT = 2 * O * 128 * 128     # BN reduction element count per channel
AF = mybir.ActivationFunctionType
ALU = mybir.AluOpType


# ---------------------------------------------------------------------------
# Host-side weight/input preparation (pure numpy)
# ---------------------------------------------------------------------------

def _rot_matrices(k, n):
    c = (k - 1) / 2.0
    mats = np.zeros((n, k * k, k * k), np.float32)
    for m in range(n):
        th = 2.0 * np.pi * m / n
        co, si = np.cos(th), np.sin(th)
        for i in range(k):
            for j in range(k):
                di, dj = i - c, j - c
                sy = co * di + si * dj + c
                sx = -si * di + co * dj + c
                i0, j0 = int(np.floor(sy)), int(np.floor(sx))
                fy, fx = sy - i0, sx - j0
                for ii, jj, w in ((i0, j0, (1 - fy) * (1 - fx)), (i0, j0 + 1, (1 - fy) * fx),
                                  (i0 + 1, j0, fy * (1 - fx)), (i0 + 1, j0 + 1, fy * fx)):
                    if 0 <= ii < k and 0 <= jj < k and w > 1e-8:
                        mats[m, i * k + j, ii * k + jj] = w
    return mats


def _prep_rotated_weights(lift_w, conv_w, w10):
    M7 = _rot_matrices(7, O)
    M5 = _rot_matrices(5, O)
    Wlift = np.zeros((O, C, 3, 7, 7), np.float32)          # [m,co,ci,dy,dx]
    for m in range(O):
        Wlift[m] = (lift_w.reshape(C, 3, 49) @ M7[m].T).reshape(C, 3, 7, 7)
    Wconv = []
    for L in range(9):
        W = conv_w[L]                                      # [co,ci,n,5,5]
        Wf = np.zeros((O, C, O, C, 5, 5), np.float32)      # [m,co,n,ci,dy,dx]
        for m in range(O):
            Wm = np.roll(W, m, axis=2)
            Wm = (Wm.reshape(C, C, O, 25) @ M5[m].T).reshape(C, C, O, 5, 5)
            Wf[m] = Wm.transpose(0, 2, 1, 3, 4)
        Wconv.append(Wf)
    W10mat = np.zeros((96, 128), np.float32)
    for m in range(O):
        Wm = np.roll(w10[:, :, :, 0, 0], m, axis=2)        # [co,ci,n]
        W10mat[:80, m * CF:(m + 1) * CF] = Wm.transpose(2, 1, 0).reshape(80, CF)
    return Wlift, Wconv, W10mat


def _pack_m16(Wd5):
    """Pack flipped rotated weights [dy][dx][K=80][M=80] into the 16-matmul
    exact-cover stationary sets (zero slot waste: 2000 = 15*128 + 80).

    Tile A slots: p<80: ch p @ dx+0 ; p>=80: ch p-80 (0-47) @ dx+1
    Tile B slots: p<32: ch 48+p @ dx+1 ; 32<=p<112: ch p-32 @ dx+2 ;
                  p>=112: ch p-112 (0-15) @ dx+3
    Tile C slots: p<64: ch 16+p @ dx+3 ; p>=64: ch p-64 (0-63) @ dx+4
    Tile E slots: p -> ch 64+p%16 @ (dy=p//16, dx=4)
    Per chunk: 5 dy x (A,B,C) at stream offset (dy, 0) + one E at (0, 4).
    """
    SABC = np.zeros((15, 128, 80), np.float32)
    SE = np.zeros((80, 80), np.float32)
    for dy in range(5):
        S = SABC[dy * 3 + 0]
        S[0:80] = Wd5[dy][0]
        S[80:128] = Wd5[dy][1][0:48]
        S = SABC[dy * 3 + 1]
        S[0:32] = Wd5[dy][1][48:80]
        S[32:112] = Wd5[dy][2]
        S[112:128] = Wd5[dy][3][0:16]
        S = SABC[dy * 3 + 2]
        S[0:64] = Wd5[dy][3][16:80]
        S[64:128] = Wd5[dy][4][0:64]
        SE[16 * dy:16 * dy + 16] = Wd5[dy][4][64:80]
    return SABC, SE


def _check_pack16(Wd5, SABC, SE):
    # reconstruct effective per-(dy, dx, K, M) weight usage and compare
    acc = np.zeros((5, 5, 80, 80), np.float32)
    for dy in range(5):
        S = SABC[dy * 3 + 0]
        acc[dy, 0] += S[0:80]
        acc[dy, 1][0:48] += S[80:128]
        S = SABC[dy * 3 + 1]
        acc[dy, 1][48:80] += S[0:32]
        acc[dy, 2] += S[32:112]
        acc[dy, 3][0:16] += S[112:128]
        S = SABC[dy * 3 + 2]
        acc[dy, 3][16:80] += S[0:64]
        acc[dy, 4][0:64] += S[64:128]
        acc[dy, 4][64:80] += SE[16 * dy:16 * dy + 16]
    ref = np.stack([np.stack([Wd5[dy][dx] for dx in range(5)]) for dy in range(5)])
    assert np.abs(acc - ref).max() < 1e-6, "M=16 packing mismatch"


def _group_mat(P, nch):
    idx = np.arange(P)
    Gm = (idx[:, None] % nch == idx[None, :] % nch).astype(np.float32)
    return Gm * (1.0 / NTOT)


def _slot_bcast():
    # B1 [80,128], B2 [80,112]: B[k, p] = 1 if slot p holds channel k
    B1 = np.zeros((80, 128), np.float32)
    B2 = np.zeros((80, 112), np.float32)
    for p in range(128):
        B1[p if p < 80 else p - 80, p] = 1.0
    for p in range(112):
        B2[48 + p if p < 32 else p - 32, p] = 1.0
    return B1, B2


def prep_in_maps(inputs):
    x = np.asarray(inputs['x'], np.float32)
    lift_w = np.asarray(inputs['lift_w'], np.float32)
    conv_w = np.asarray(inputs['conv_w'], np.float32)
    w10 = np.asarray(inputs['w10'], np.float32)
    wfinal = np.asarray(inputs['wfinal'], np.float32)

    Wlift, Wconv, W10mat = _prep_rotated_weights(lift_w, conv_w, w10)

    # 128 rows: slots 80-127 of tile A hold ch (p-80), and p-80 = p mod 10
    # (80 = 0 mod 10), so the modular formula extends unchanged
    gb = np.zeros((128, 10, 2), np.float32)
    gb[:, 0, 0] = np.asarray(inputs['lift_g'], np.float32)[np.arange(128) % C]
    gb[:, 0, 1] = np.asarray(inputs['lift_b'], np.float32)[np.arange(128) % C]
    cg = np.asarray(inputs['conv_g'], np.float32)
    cb = np.asarray(inputs['conv_b'], np.float32)
    for L in range(9):
        gb[:, L + 1, 0] = cg[L][np.arange(128) % C]
        gb[:, L + 1, 1] = cb[L][np.arange(128) % C]
    gb10 = np.zeros((128, 2), np.float32)
    gb10[:, 0] = np.asarray(inputs['g10'], np.float32)[np.arange(128) % CF]
    gb10[:, 1] = np.asarray(inputs['b10'], np.float32)[np.arange(128) % CF]

    G = ((np.arange(80)[:, None] % C == np.arange(128)[None, :] % C)
         .astype(np.float32) * (1.0 / NTOT))        # [K=80, M=128]
    G10 = _group_mat(128, CF)
    # final 1x1 conv as a block-selector: out[b, n] = sum_c w[c]*m128[16b+c, n]
    wf = wfinal[0, :, 0, 0]
    wfin = np.zeros((128, 8), np.float32)
    wfin[np.arange(128), np.arange(128) // 16] = wf[np.arange(128) % 16]
    wfin = wfin.astype(NPDT)

    # per-(qy,qx) flipped weight variants
    wvar = {}
    for qy in range(2):
        for qx in range(2):
            fy = slice(None, None, -1) if qy else slice(None)
            fx = slice(None, None, -1) if qx else slice(None)
            Wl = Wlift[:, :, :, fy, fx]                    # [m,co,ci,dy,dx]
            liftWmat = np.zeros((192, 80), np.float32)     # 147 rows + K-pad
            liftWmat[:147] = (
                Wl.transpose(3, 4, 2, 0, 1).reshape(49, 3, 80).reshape(147, 80))
            W1 = np.zeros((9, 128, 15, 80), np.float32)   # [L, K, d, M]
            W2 = np.zeros((9, 80, 80), np.float32)        # E tile weights
            for L in range(9):
                Wf = Wconv[L][:, :, :, :, fy, fx]          # [m,co,n,ci,dy,dx]
                Wd5 = Wf.transpose(4, 5, 2, 3, 0, 1).reshape(5, 5, 80, 80)
                SABC, SE = _pack_m16(Wd5)
                if L == 0:
                    _check_pack16(Wd5, SABC, SE)
                W1[L] = SABC.transpose(1, 0, 2)
                W2[L] = SE
            wvar[(qy, qx)] = (liftWmat.astype(NPDT), W1.astype(NPDT),
                              W2.astype(NPDT))

    in_maps = []
    cores = []
    for b in range(2):
        for qy in range(2):
            for qx in range(2):
                cores.append((b, qy, qx))
                xf = x[b]
                if qy:
                    xf = xf[:, ::-1, :]
                if qx:
                    xf = xf[:, :, ::-1]
                win = xf[:, 0:85, 0:85]
                xw = np.pad(win, ((0, 0), (3, 0), (3, 0)), mode='reflect')
                col = np.zeros((192, E0 * E0), NPDT)       # 147 rows + K-pad
                for t in range(49):
                    dy, dx = t // 7, t % 7
                    col[t * 3:t * 3 + 3] = (
                        xw[:, dy:dy + E0, dx:dx + E0].reshape(3, -1))
                liftWmat, W1, W2 = wvar[(qy, qx)]
                in_maps.append({
                    'xcol1': np.ascontiguousarray(col[:96]),
                    'xcol2': np.ascontiguousarray(col[96:]),
                    'liftW1': np.ascontiguousarray(liftWmat[:96]),
                    'liftW2': np.ascontiguousarray(liftWmat[96:]),
                    'convW1': W1,
                    'convW2': W2,
                    'w10': W10mat.astype(NPDT),
                    'wfin': wfin,
                    'G': G,
                    'G10': G10,
                    'gb': gb,
                    'gb10': gb10,
                })
    return in_maps, cores


# ---------------------------------------------------------------------------
# Bass program
# ---------------------------------------------------------------------------

def build_program(no_cc=None, nconv=None, fold_max=True):
    if no_cc is None:
        no_cc = bool(os.environ.get("GK_NO_CC"))
    if nconv is None:
        nconv = int(os.environ.get("GK_NCONV", "9"))
    use_ag = os.environ.get("GK_CC", "ag") == "ag"
    nc = bacc.Bacc("TRN2", target_bir_lowering=False, debug=False,
                   enable_asserts=False, num_devices=8)

    d_xcol1 = nc.dram_tensor("xcol1", [96, E0 * E0], F16, kind="ExternalInput")
    d_xcol2 = nc.dram_tensor("xcol2", [96, E0 * E0], F16, kind="ExternalInput")
    d_liftW1 = nc.dram_tensor("liftW1", [96, 80], F16, kind="ExternalInput")
    d_liftW2 = nc.dram_tensor("liftW2", [96, 80], F16, kind="ExternalInput")
    d_convW1 = nc.dram_tensor("convW1", [9, 128, 15, 80], F16, kind="ExternalInput")
    d_convW2 = nc.dram_tensor("convW2", [9, 80, 80], F16, kind="ExternalInput")
    d_w10 = nc.dram_tensor("w10", [96, 128], F16, kind="ExternalInput")
    d_wfin = nc.dram_tensor("wfin", [128, 8], F16, kind="ExternalInput")
    d_G = nc.dram_tensor("G", [80, 128], F32, kind="ExternalInput")
    d_G10 = nc.dram_tensor("G10", [128, 128], F32, kind="ExternalInput")
    d_gb = nc.dram_tensor("gb", [128, 10, 2], F32, kind="ExternalInput")
    d_gb10 = nc.dram_tensor("gb10", [128, 2], F32, kind="ExternalInput")
    d_out = nc.dram_tensor("out", [1, 64 * 64], F32, kind="ExternalOutput")

    rg = [list(range(8))]

    with tile.TileContext(nc) as tc:
        with (
            tc.tile_pool(name="const", bufs=1) as constp,
            tc.tile_pool(name="acts", bufs=2) as actsp,
            tc.tile_pool(name="tails", bufs=1) as tailsp,
            tc.tile_pool(name="smallp", bufs=8) as smallp,
            tc.tile_pool(name="sbp", bufs=6) as sbp,
            tc.tile_pool(name="sqp", bufs=1) as sqp,
            tc.tile_pool(name="pscv", bufs=6, space="PSUM") as pscv,
            tc.tile_pool(name="psst", bufs=2, space="PSUM") as psst,
            tc.tile_pool(name="dram", bufs=2, space="DRAM") as dramp,
        ):
            # ---- sigmoid table preload (pays ACT_TABLE_LOAD during startup) ----
            epst = constp.tile([128, 1], F32, tag="eps")
            nc.vector.memset(epst[:], EPS)
            sigwarm = smallp.tile([1, 1], F32, tag="sigwarm")
            nc.scalar.activation(sigwarm[:], epst[0:1, 0:1], AF.Sigmoid)
            if os.environ.get("GK_WARMUP") and not no_cc:
                wu_in = dramp.tile([8, 1], F32, tag="cc_in")
                wu_out = dramp.tile([8, 1], F32, tag="cc_out")
                nc.gpsimd.dma_start(out=wu_in[:], in_=epst[0:8, 0:1])
                nc.gpsimd.collective_compute(
                    "AllReduce", ALU.add, replica_groups=rg,
                    ins=[wu_in[:].opt()], outs=[wu_out[:].opt()])

            # ---- load constants; xcol chunked so lift starts early ----
            r0 = 512 // E0                       # lift chunk rows (6)
            lift_chunks = [(y0, min(r0, E0 - y0)) for y0 in range(0, E0, r0)]
            # xcol is dead after the lift; borrow the tail-phase buffers
            # (tile framework inserts WAR sync on tag reuse)
            xc1 = tailsp.tile([96, E0 * E0], F16, tag="b0")
            xc2 = tailsp.tile([96, E0 * E0], F16, tag="m128")
            # first 5 lift-chunks worth of xcol, then the rest
            splits = [0, 5 * r0 * E0, E0 * E0]
            for a, b in zip(splits[:-1], splits[1:]):
                nc.sync.dma_start(out=xc1[:, a:b], in_=d_xcol1.ap()[:, a:b])
                nc.sync.dma_start(out=xc2[:, a:b], in_=d_xcol2.ap()[:, a:b])
            lw1 = constp.tile([96, 80], F16, tag="lw1")
            nc.sync.dma_start(out=lw1[:], in_=d_liftW1.ap())
            lw2 = constp.tile([96, 80], F16, tag="lw2")
            nc.sync.dma_start(out=lw2[:], in_=d_liftW2.ap())
            Gt = constp.tile([80, 128], F32, tag="G")
            nc.sync.dma_start(out=Gt[:], in_=d_G.ap())
            gbt = constp.tile([128, 10, 2], F32, tag="gb")
            nc.sync.dma_start(out=gbt[:], in_=d_gb.ap())
            cw1 = []
            cw2 = []
            for L in range(9):
                t1w = constp.tile([128, 15, 80], F16, tag=f"cw1_{L}")
                nc.sync.dma_start(out=t1w[:], in_=d_convW1.ap()[L])
                t2w = constp.tile([80, 80], F16, tag=f"cw2_{L}")
                nc.sync.dma_start(out=t2w[:], in_=d_convW2.ap()[L])
                cw1.append(t1w)
                cw2.append(t2w)
            G10t = constp.tile([128, 128], F32, tag="G10")
            nc.sync.dma_start(out=G10t[:], in_=d_G10.ap())
            gb10t = constp.tile([128, 2], F32, tag="gb10")
            nc.sync.dma_start(out=gb10t[:], in_=d_gb10.ap())
            w10t = constp.tile([96, 128], F16, tag="w10")
            nc.sync.dma_start(out=w10t[:], in_=d_w10.ap())
            wfint = constp.tile([128, 8], F16, tag="wfin")
            nc.sync.dma_start(out=wfint[:], in_=d_wfin.ap())

            def collective(v_ap, P):
                """Launch stats collective on [P,2]; returns fn() -> sbuf [P,2].

                cin/fetch DMAs ride the gpsimd queue so the CC doorbell's
                semaphore wait is not entangled with bulk DMA traffic."""
                cin = dramp.tile([P, 2], F32, tag="cc_in")
                nc.gpsimd.dma_start(out=cin[:], in_=v_ap)
                if no_cc:
                    cout = dramp.tile([P, 2], F32, tag="cc_out")
                    nc.gpsimd.dma_start(out=cout[:], in_=cin[:])

                    def fetch():
                        post = smallp.tile([P, 2], F32, tag="post")
                        nc.gpsimd.dma_start(out=post[:], in_=cout[:])
                        return post
                elif use_ag:
                    cout = dramp.tile([8, P, 2], F32, tag="cc_out")
                    nc.gpsimd.collective_compute(
                        "AllGather", ALU.bypass, replica_groups=rg,
                        ins=[cin[:].opt()], outs=[cout[:].opt()])

                    def fetch():
                        ga = smallp.tile([P, 2, 8], F32, tag="ga")
                        nc.gpsimd.dma_start(
                            out=ga[:], in_=cout[:].rearrange("g c s -> c s g"))
                        post = smallp.tile([P, 2], F32, tag="post")
                        nc.vector.tensor_reduce(
                            out=post[:], in_=ga[:],
                            axis=mybir.AxisListType.X, op=ALU.add)
                        return post
                else:
                    cout = dramp.tile([P, 2], F32, tag="cc_out")
                    nc.gpsimd.collective_compute(
                        "AllReduce", ALU.add, replica_groups=rg,
                        ins=[cin[:].opt()], outs=[cout[:].opt()])

                    def fetch():
                        post = smallp.tile([P, 2], F32, tag="post")
                        nc.gpsimd.dma_start(out=post[:], in_=cout[:])
                        return post
                return fetch

            def stats_chain(post, P, Gtile, gamma_ap, beta_ap, Po=None):
                """post [P,2] raw sums -> (scl, bia) [Po,1] each (SBUF)."""
                Po = P if Po is None else Po
                P = Po
                gps = psst.tile([P, 2], F32, tag="st")
                nc.tensor.matmul(gps[:], Gtile[:], post[:],
                                 start=True, stop=True)
                msq = smallp.tile([P, 1], F32, tag="msq")
                nc.scalar.square(msq[:], gps[:, 0:1])
                var = smallp.tile([P, 1], F32, tag="var")
                nc.vector.tensor_sub(var[:], gps[:, 1:2], msq[:])
                std = smallp.tile([P, 1], F32, tag="std")
                nc.scalar.activation(std[:], var[:], AF.Sqrt,
                                     bias=epst[0:P, 0:1])
                rstd = smallp.tile([P, 1], F32, tag="rstd")
                nc.vector.reciprocal(rstd[:], std[:])
                sclbia = sbp.tile([P, 2], F32, tag="sclbia")
                nc.vector.tensor_mul(sclbia[:, 0:1], rstd[:], gamma_ap)
                tb = smallp.tile([P, 1], F32, tag="tb")
                nc.vector.tensor_mul(tb[:], gps[:, 0:1], sclbia[:, 0:1])
                nc.vector.tensor_sub(sclbia[:, 1:2], beta_ap, tb[:])
                return sclbia

            # ================= layer emitters =================
            state = {}

            def emit_packed_layer(mms_fn, E_out, lidx, plain_out=False):
                """Emit one conv-like layer writing frame tiles (or plain buf).

                mms_fn(ps, y0, rr): accumulate matmuls for out rows [y0,y0+rr).
                Output written to tile-A primary frame at (2+y, 2+x) (or buf
                at (y, x) when plain_out).  Leaves state['A'..'E'] (or
                state['buf']) for the next layer.
                """
                r = 512 // E_out
                # tiny first chunk: needs only rows [0,6) applied, so the
                # PE restarts ~4us earlier after each stats collective
                chunks = [(0, 2)] + [(y0, min(r, E_out - y0))
                                     for y0 in range(2, E_out, r)]
                img_chunks = [c for c in chunks if c[0] < 64]
                mar_chunks = [c for c in chunks if c[0] >= 64]
                n_img = len(img_chunks)
                doff = 0 if plain_out else 2
                # next-layer input frame extent = E_out + 2 borders
                Fn = E_out + 2
                if plain_out:
                    dst = tailsp.tile([96, 64, 64], F16, tag="buf9")
                    nc.vector.memset(dst[64:96, :, :], 0.0)
                    state['buf'] = dst
                    tA = dst
                else:
                    tA = actsp.tile([128, Fn, Fn], F16, tag="tA")
                    nc.vector.memset(tA[64:128, Fn - 1:Fn, Fn - 1:Fn], 0.0)
                    tB = actsp.tile([128, Fn, Fn], F16, tag="tB")
                    tC = actsp.tile([128, Fn, Fn], F16, tag="tC")
                    tE = actsp.tile([80, Fn, Fn], F16, tag="tE")
                    state['A'], state['B'] = tA, tB
                    state['C'], state['E'] = tC, tE

                parts_s = smallp.tile([80, 16], F32, tag="pts")
                parts_q = smallp.tile([80, 16], F32, tag="ptq")

                def do_chunk(ci, y0, rr, stats):
                    ps = pscv.tile([80, rr, E_out], F32, tag="cv")
                    mms_fn(ps, y0, rr)
                    nc.scalar.copy(
                        tA[0:80, doff + y0:doff + y0 + rr,
                           doff:doff + E_out], ps[:])
                    if not plain_out:
                        # col reflect (raw): cols 0,1 <- 4,3 for these rows
                        rows = slice(doff + y0, doff + y0 + rr)
                        nc.vector.tensor_copy(tA[0:80, rows, 0:1],
                                              tA[0:80, rows, 4:5])
                        nc.vector.tensor_copy(tA[0:80, rows, 1:2],
                                              tA[0:80, rows, 3:4])
                        if ci == 1:
                            # row reflect (raw): rows 0,1 <- 4,3 full width
                            # (frame row 4 is first written by chunk 1)
                            nc.vector.tensor_copy(tA[0:80, 0:1, :],
                                                  tA[0:80, 4:5, :])
                            nc.vector.tensor_copy(tA[0:80, 1:2, :],
                                                  tA[0:80, 3:4, :])
                    if stats:
                        h = min(y0 + rr, 64) - y0
                        reg = tA[0:80, doff + y0:doff + y0 + h,
                                 doff:doff + 64]
                        nc.vector.tensor_reduce(
                            out=parts_s[:, ci:ci + 1], in_=reg,
                            axis=mybir.AxisListType.XY, op=ALU.add)
                        sqc = sqp.tile([80, 8, 64], F16, tag="sqc")
                        nc.vector.tensor_mul(sqc[0:80, 0:h, :], reg, reg)
                        nc.vector.tensor_reduce(
                            out=parts_q[:, ci:ci + 1],
                            in_=sqc[0:80, 0:h, :],
                            axis=mybir.AxisListType.XY, op=ALU.add)

                # flat views: a column shift s becomes a flat offset of s
                # elements, so every dup copy is one contiguous segment per
                # partition (single DMA packet instead of per-row packets).
                # Wrapped elements land only in cols >= Fn-4, never streamed.
                fA = tA[:].rearrange("p r c -> p (r c)") if not plain_out \
                    else None
                if not plain_out:
                    fB = tB[:].rearrange("p r c -> p (r c)")
                    fC = tC[:].rearrange("p r c -> p (r c)")
                    fE = tE[:].rearrange("p r c -> p (r c)")

                def dup_band(ra, rb):
                    # duplicate APPLIED rows [ra, rb) of the new frame into
                    # the shifted slots (partition-moving SBUF->SBUF DMAs),
                    # spread across queues.  Copy length is clamped to the
                    # band end so no dup reads into the next relu band; the
                    # missing tail lands in cols >= Fn-4, never streamed.
                    a0 = ra * Fn
                    b0 = rb * Fn

                    def flat(q, dstf, p0, p1, sp0, sft):
                        q.dma_start(
                            out=dstf[p0:p1, a0:b0 - sft],
                            in_=fA[sp0:sp0 + (p1 - p0), a0 + sft:b0])

                    flat(nc.sync, fB, 32, 112, 0, 2)      # B g2: ch0-79 @+2
                    flat(nc.sync, fC, 64, 128, 0, 4)      # C g2: ch0-63 @+4
                    flat(nc.sync, fC, 0, 64, 16, 3)       # C g1: ch16-79 @+3
                    flat(nc.gpsimd, fB, 0, 32, 48, 1)     # B g1: ch48-79 @+1
                    flat(nc.gpsimd, fB, 112, 128, 0, 3)   # B g3: ch0-15 @+3
                    # E: row-shifted copies of ch64-79 (lags 4 applied rows)
                    ea, eb = max(0, ra - 4), max(0, rb - 4)
                    if rb >= Fn:
                        eb = Fn - 4          # final band: finish E fully
                    if eb > ea:
                        for g in range(5):
                            q = (nc.gpsimd, nc.sync, nc.scalar,
                                 nc.gpsimd, nc.scalar)[g]
                            q.dma_start(
                                out=fE[16 * g:16 * g + 16,
                                       ea * Fn:eb * Fn],
                                in_=fA[64:80,
                                       (ea + g) * Fn:(eb + g) * Fn])

                for ci, (y0, rr) in enumerate(img_chunks):
                    do_chunk(ci, y0, rr, True)
                # finalize local stats and launch the collective
                v = smallp.tile([80, 2], F32, tag="v")
                nc.vector.tensor_reduce(
                    out=v[:, 0:1], in_=parts_s[:, 0:n_img],
                    axis=mybir.AxisListType.X, op=ALU.add)
                nc.vector.tensor_reduce(
                    out=v[:, 1:2], in_=parts_q[:, 0:n_img],
                    axis=mybir.AxisListType.X, op=ALU.add)
                fetch = collective(v[:], 80)
                # margin chunks cover the collective latency on the PE
                for ci, (y0, rr) in enumerate(mar_chunks):
                    do_chunk(n_img + ci, y0, rr, False)
                # keep-warm dummy matmuls: keep the PE HAM throttle released
                # while waiting for the collective (no reader, no drain)
                post = fetch()
                sclbia = stats_chain(post, 80, Gt,
                                     gbt[:, lidx, 0:1], gbt[:, lidx, 1:2],
                                     Po=128)
                if plain_out:
                    state['sb9'] = sclbia
                    return
                state['apply'] = (tA, dup_band, sclbia, Fn)

            def apply_rows(ra, rb):
                tA, dup_band, sclbia, Fn = state['apply']
                rb = min(rb, Fn)
                if rb <= ra:
                    return
                # A-dup copies RAW rows (whole raw frame exists before any
                # apply), so it executes during the collective wait; the relu
                # then covers primary+dup slots in one 128-partition op
                fA = tA[:].rearrange("p r c -> p (r c)")
                a0, b0 = ra * Fn, rb * Fn
                ln = min(b0 + 1, Fn * Fn) - (a0 + 1)
                nc.scalar.dma_start(out=fA[80:128, a0:a0 + ln],
                                    in_=fA[0:48, a0 + 1:a0 + 1 + ln])
                nc.scalar.activation(tA[0:128, ra:rb, :],
                                     tA[0:128, ra:rb, :], AF.Relu,
                                     bias=sclbia[:, 1:2],
                                     scale=sclbia[:, 0:1])
                dup_band(ra, rb)

            def apply_rows9(ra, rb):
                buf, sclbia = state['buf'], state['sb9']
                rb = min(rb, 64)
                if rb <= ra:
                    return
                nc.scalar.activation(buf[0:80, ra:rb, :],
                                     buf[0:80, ra:rb, :], AF.Relu,
                                     bias=sclbia[0:80, 1:2],
                                     scale=sclbia[0:80, 0:1])

            # ---- lift layer ----
            def lift_mms(ps, y0, rr):
                nc.tensor.matmul(ps[:], lw1[:],
                                 xc1[:, y0 * E0:(y0 + rr) * E0],
                                 start=True, stop=False)
                nc.tensor.matmul(ps[:], lw2[:],
                                 xc2[:, y0 * E0:(y0 + rr) * E0],
                                 start=False, stop=True)

            emit_packed_layer(lift_mms, E0, 0)

            # ---- conv layers 1..9 ----
            prev_apply = apply_rows
            for L in range(9):
                E_in = E0 - 2 * L            # input extent (incl borders = E_in+2)
                E_out = E_in - 2
                plain = (L == 8)
                sA, sB = state['A'], state['B']
                sC, sE = state['C'], state['E']
                w1, w2 = cw1[L], cw2[L]

                def conv_mms(ps, y0, rr, w1=w1, w2=w2, sA=sA, sB=sB,
                             sC=sC, sE=sE, E_out=E_out):
                    for d in range(15):
                        dy, j = d // 3, d % 3
                        src = (sA, sB, sC)[j]
                        nc.tensor.matmul(
                            ps[:], w1[:, d, :],
                            src[0:128, y0 + dy:y0 + dy + rr, 0:E_out],
                            start=(d == 0), stop=False)
                    nc.tensor.matmul(
                        ps[:], w2[:],
                        sE[0:80, y0:y0 + rr, 4:4 + E_out],
                        start=False, stop=True)

                # interleave previous layer's apply with this layer's chunks:
                # emit_packed_layer internally iterates chunks; we need the
                # apply of the PREVIOUS layer to be emitted piecewise before
                # each chunk.  Wrap mms_fn to emit apply pieces lazily.
                r = 512 // E_out
                need = {'done': 0}

                def conv_mms_gated(ps, y0, rr, conv_mms=conv_mms, r=r,
                                   need=need, L=L):
                    # ensure prev apply covers rows [0, y0+rr+4); after the
                    # first (tiny) band, apply one chunk ahead so dup DMAs
                    # batch over ~2-chunk bands
                    if prev_apply is not None:
                        want = y0 + rr + 4
                        if want > need['done']:
                            if need['done'] > 0:
                                want += rr
                            prev_apply(need['done'], want)
                            need['done'] = want
                    conv_mms(ps, y0, rr)

                emit_packed_layer(conv_mms_gated, E_out, L + 1,
                                  plain_out=plain)
                # finish any remaining apply rows of the previous layer
                if prev_apply is not None:
                    prev_apply(need['done'], E_in + 2)
                if plain:
                    prev_apply = apply_rows9
                else:
                    prev_apply = apply_rows
                if L + 1 >= nconv:
                    pass

            # ---- conv10 (1x1, 80 -> 128 partitions) ----
            act10 = tailsp.tile([128, 64, 64], F16, tag="act10")
            buf9 = state['buf']

            need9 = {'done': 0}

            def c10_mms(ps, y0, rr, src=buf9):
                want = y0 + rr
                if want > need9['done']:
                    prev_apply(need9['done'], want)
                    need9['done'] = want
                nc.tensor.matmul(ps[:], w10t[:], src[:, y0:y0 + rr, 0:64],
                                 start=True, stop=True)

            # conv10 layer: special emit (128 partitions, no margins)
            r = 512 // 64
            chunks10 = [(y0, 8) for y0 in range(0, 64, 8)]
            parts_s = smallp.tile([128, 16], F32, tag="pts10")
            parts_q = smallp.tile([128, 16], F32, tag="ptq10")
            a10 = act10[:].rearrange("p a b -> p (a b)")
            b0 = tailsp.tile([64, 4096], F16, tag="b0")
            for ci, (y0, rr) in enumerate(chunks10):
                ps = pscv.tile([128, rr, 64], F32, tag="cv")
                c10_mms(ps, y0, rr)
                nc.scalar.copy(act10[:, y0:y0 + rr, :], ps[:])
                nc.vector.tensor_reduce(
                    out=parts_s[:, ci:ci + 1],
                    in_=act10[:, y0:y0 + rr, :],
                    axis=mybir.AxisListType.XY, op=ALU.add)
                sqc = sqp.tile([128, 8, 64], F16, tag="sqc10")
                nc.gpsimd.tensor_mul(sqc[:, 0:rr, :],
                                     act10[:, y0:y0 + rr, :],
                                     act10[:, y0:y0 + rr, :])
                nc.vector.tensor_reduce(
                    out=parts_q[:, ci:ci + 1], in_=sqc[:, 0:rr, :],
                    axis=mybir.AxisListType.XY, op=ALU.add)
                # max-tree level 1 for this chunk's pixel block (the
                # remaining levels run under the stats collective)
                csl = slice(64 * y0, 64 * (y0 + rr))
                nc.sync.dma_start(out=b0[0:64, csl], in_=a10[64:128, csl])
                nc.vector.tensor_max(a10[0:64, csl], a10[0:64, csl],
                                     b0[0:64, csl])
            v10 = smallp.tile([128, 2], F32, tag="v10")
            nc.vector.tensor_reduce(
                out=v10[:, 0:1], in_=parts_s[:, 0:len(chunks10)],
                axis=mybir.AxisListType.X, op=ALU.add)
            nc.vector.tensor_reduce(
                out=v10[:, 1:2], in_=parts_q[:, 0:len(chunks10)],
                axis=mybir.AxisListType.X, op=ALU.add)
            fetch10 = collective(v10[:], 128)

            # ---- max tree levels 2-3 (level 1 ran per conv10 chunk) ----
            # in-place in act10 with one scratch tile to save SBUF; levels
            # split into halves so the copy/max ping-pong pipelines
            for p in (32, 16):
                for h in range(2):
                    cs = slice(2048 * h, 2048 * (h + 1))
                    nc.sync.dma_start(out=b0[0:p, cs],
                                      in_=a10[p:2 * p, cs])
                    nc.vector.tensor_max(a10[0:p, cs], a10[0:p, cs],
                                         b0[0:p, cs])
            # spread [16, 8*512] -> [128, 512]: partition 16*b+c holds
            # channel c, column block b (still under the AR)
            m128 = tailsp.tile([128, 512], F16, tag="m128")
            for b in range(8):
                q = (nc.scalar, nc.sync)[b % 2]
                q.dma_start(out=m128[16 * b:16 * b + 16, :],
                            in_=a10[0:16, 512 * b:512 * (b + 1)])

            post10 = fetch10()
            sb10 = stats_chain(post10, 128, G10t,
                               gb10t[:, 0:1], gb10t[:, 1:2])
            # relu(s*h+b) wide (gamma>0 commutes w/ max), one final matmul
            # with the block-selector weights, one sigmoid, one output DMA
            nc.scalar.activation(m128[:], m128[:], AF.Relu,
                                 bias=sb10[:, 1:2], scale=sb10[:, 0:1])
            fp = psst.tile([8, 512], F32, tag="st")
            nc.tensor.matmul(fp[:], wfint[:], m128[:],
                             start=True, stop=True)
            ob = smallp.tile([8, 512], F32, tag="outsb")
            nc.scalar.activation(ob[:], fp[:], AF.Sigmoid)
            nc.sync.dma_start(
                out=d_out.ap().rearrange("o (b n) -> (o b) n", b=8),
                in_=ob[:])

    nc.compile()
    return nc


_CACHED = {}


def _get_program(fold_max):
    key = ('nc', fold_max)
    if key not in _CACHED:
        _CACHED[key] = build_program(fold_max=fold_max)
    return _CACHED[key]


LAST = None


def kernel(**inputs):
    global LAST
    in_maps, cores = prep_in_maps(inputs)
    fold_max = bool((np.asarray(inputs['g10'], np.float32) > 0).all())
    nc = _get_program(fold_max)
    res = bass_utils.run_bass_kernel_spmd(
        nc, in_maps, core_ids=list(range(8)),
        trace=bool(os.environ.get("GK_TRACE")))
    LAST = res
    out = np.zeros((2, 1, 128, 128), np.float32)
    for (core, omap) in zip(cores, res.results):
        b, qy, qx = core
        q = np.asarray(omap['out'], np.float32).reshape(64, 64)
        if qy:
            q = q[::-1, :]
        if qx:
            q = q[:, ::-1]
        out[b, 0, qy * 64:(qy + 1) * 64, qx * 64:(qx + 1) * 64] = q
    return out

